# revision 33
# baseline (speedup 1.0000x reference)
"""Trainium2 Bass kernel for nn_ConvexReLUCNN.

Math (identical multilinear form as the reference, reordered):
    reference:  U = unfold(x,3); A = U.G^T (54 GFLOP); out = A.(v-w)
    here:       CS[(q,z),(dh,s)] = sum_{m,p} pd[m,(p,q,z)] * G[m,(dh+2-p,s)]
                     (one 12-matmul PSUM accumulation; i-shift absorbed in
                      shifted G windows, p-sum absorbed in the accumulation)
                W^T[(dh,w), z]   = sum_q CS[(q,z), (dh, w-q)]
                     (12 tiny transpose-matmuls vs block identity J;
                      q-shift absorbed in zero-padded CT column views)
                out^T[o, b]      = sum_chunks W_chunk^T @ x^T_chunk

Distribution: sharded by image row band. Core i owns output-image rows
h in [8i, 8i+8) (all channels, widths, batches); host sums the 8 partial
outputs (tiny) - no device collectives.

Schedule: DMA FIFO = (pd|J|g_t01) | g_t23 | x in 4 chunks [5,4,2,1], so
CS starts as the first G half lands and the final matmuls chase the x
chunks, with a tiny last chunk to shorten the x-gated tail. PE junk
warmup (12 wide + 3 taper) bridges the G wait - sized for slow-DMA runs
(cross-core HBM contention varies run to run) - so the HAM clock gate is
at full rate when CS starts - any PE gap >0.5us mid-kernel downclocks
the PE to 1.2 GHz for several microseconds. ONE 96-partition ACT cast
moves CS psum->SBUF (DVE psum-fp32->bf16 casts corrupt on HW). The
transpose phase uses 4 separate PSUM tiles so each jb block's WsB cast
(ACT) overlaps the next block's matmuls. All matmul operands sit at
SBUF base partition 0 (non-zero base partitions hang real HW), with
eye(96) column blocks masking the q selection. bf16 output, host
upcasts and sums partials in float64.

All wire data is bf16, prepared host-side:
  - xt:  x band pre-transposed to [chw=1536, b=512] -> [128, 12*512]
  - gv:  pd=(v-w) pre-permuted to the padded (p,q,z=c*10+o) layout
         [128, 1152], block identity J = eye(96), G band rows t=0,1
  - g2:  G band rows t=2,3 (zero-clipped halo)
"""

import numpy as np
from contextlib import ExitStack

import ml_dtypes

import concourse.bass as bass
import concourse.mybir as mybir
import concourse.tile as tile
from concourse import bacc
from concourse.bass_utils import run_bass_kernel_spmd

N_CORES = 8
B_FULL = 512
C_CH, H, W = 3, 64, 64
HB = H // N_CORES           # 8 image rows per core
BAND = C_CH * HB * W        # 1536 chw positions per core
M = 512                     # num_neurons
O = 10
Ho = Wo = 62
IW = HB + 2                 # 10 patch-grid rows feeding one band
NL = IW * Wo                # 620 local G columns
Z = 32                      # padded (c,o) block: 3*10 -> 32
KO2 = 9 * Z                 # 288 = 3p x 3q x 32z
NW = HB * Wo                # 496 = shifted-G window (8 rows x 62)
RW = 64                     # CT row width: payload s=0..62 at cols 2..64,
                            # so the per-q shifted [96,128] lhsT window is a
                            # flat 1D slice (row wrap hits border zeros)
CTS_LEN = HB * RW + 2       # 514 (+2 tail zeros for the q=0 wrap past row 7)
VW_LEN = 4 * KO2            # 1152 (host-subtracted pd = v - w)
VWJ_LEN = VW_LEN + 3 * Z    # 1248 (+ block identity J = eye(96))
GV_LEN = VWJ_LEN + 2 * NL   # 2488 (pd | J | g t=0,1)
XSPLIT = [5, 4, 3]          # t12 tiles per x chunk: fewer transfers beat
                            # finer tail-gating (each extra DMA costs ~0.6us
                            # of stream time; final phase is PE-bound anyway)
NWARM_WIDE = 12             # wide (512-col) PE warmup matmuls
NWARM_TAPER = 3             # narrow (128-col) taper warmups

F32 = mybir.dt.float32
BF16 = mybir.dt.bfloat16
BF16NP = ml_dtypes.bfloat16

_NC = None


def _build():
    nc = bacc.Bacc("TRN2", target_bir_lowering=False, debug=False,
                   num_devices=N_CORES)
    x_d = nc.dram_tensor("xt", [128, 12 * B_FULL], BF16,
                         kind="ExternalInput").ap()
    gv_d = nc.dram_tensor("gv", [128, GV_LEN], BF16,
                          kind="ExternalInput").ap()
    g2_d = nc.dram_tensor("g2", [128, 2 * NL], BF16,
                          kind="ExternalInput").ap()
    o_d = nc.dram_tensor("out", [O, B_FULL], BF16, kind="ExternalOutput").ap()

    with tile.TileContext(nc) as tc, ExitStack() as ctx:
        const = ctx.enter_context(tc.tile_pool(name="const", bufs=1))
        big = ctx.enter_context(tc.tile_pool(name="big", bufs=1))
        psW = ctx.enter_context(tc.tile_pool(name="psW", bufs=1, space="PSUM"))
        psC = ctx.enter_context(tc.tile_pool(name="psC", bufs=1, space="PSUM"))
        psT = ctx.enter_context(tc.tile_pool(name="psT", bufs=1, space="PSUM"))
        psF = ctx.enter_context(tc.tile_pool(name="psF", bufs=1, space="PSUM"))

        # ---- DMA triggers first: all on the sync HWDGE ring, FIFO --------
        # FIFO order == consumption order: (pd|J|g_t0), g_t1, g_t23, x.
        gvt = big.tile([128, GV_LEN], BF16, tag="gvt")
        nc.sync.dma_start(gvt[:], gv_d)
        g2t = big.tile([128, 2, NL], BF16, tag="g2t")
        nc.sync.dma_start(g2t[:], g2_d)
        XT = [big.tile([128, n, B_FULL], BF16, tag=f"X{j}", name=f"X{j}")
              for j, n in enumerate(XSPLIT)]
        w0 = 0
        for j, n in enumerate(XSPLIT):
            nc.sync.dma_start(XT[j][:], x_d[:, w0:w0 + n * B_FULL])
            w0 += n * B_FULL

        # ---- memsets: junk on GpSimd (earliest engine up -> PE warmup
        # starts ~1.5us sooner, so the HAM clock gate hits full rate by CS)
        junk = const.tile([128, 512], BF16)
        nc.gpsimd.memset(junk[:], 0.25)
        # CT: rows (q,z), cols (dh, 2+s) with 64-wide rows, zero borders
        CT = const.tile([96, CTS_LEN], BF16, name="CT")
        nc.vector.memset(CT[:], 0.0)

        # ---- PE p-state warmup on junk (bridges the vwj/g1 DMA wait) -----
        pj = psW.tile([128, 512], F32, tag="pj")
        for _ in range(NWARM_WIDE):
            nc.tensor.matmul(pj[:], junk[:, 0:128], junk[:, 0:512],
                             start=True, stop=True)
        for _ in range(NWARM_TAPER):
            nc.tensor.matmul(pj[:, 0:128], junk[:, 0:128], junk[:, 0:128],
                             start=True, stop=True)

        # pd = v - w arrives host-subtracted in the padded (p, q, z) layout
        pd2 = gvt[:, 0:VW_LEN].rearrange("p (t k) -> p t k", t=4)
        Jt = gvt[:, VW_LEN:VWJ_LEN]             # [96 used, 96] = eye(96)
        gs1 = gvt[:, VWJ_LEN:GV_LEN].rearrange("p (t k) -> p t k", t=2)

        # ---- CS = sum_{t,p} pd2_tp.T @ G_t[rows 2-p .. 10-p] -------------
        ps = psC.tile([96, NW], F32, tag="psC")
        gblks = [gs1[:, 0, :], gs1[:, 1, :], g2t[:, 0, :], g2t[:, 1, :]]
        for t in range(4):
            gblk = gblks[t]
            for p in range(3):
                nc.tensor.matmul(
                    ps[:],
                    pd2[:, t, 96 * p:96 * (p + 1)],
                    gblk[:, Wo * (2 - p):Wo * (2 - p) + NW],
                    start=(t == 0 and p == 0), stop=(t == 3 and p == 2))
        psv = ps[:].rearrange("p (h s) -> p h s", s=Wo)
        # ONE 96-partition psum->SBUF cast (DVE psum-fp32->bf16 corrupts on
        # HW; ACT is the safe engine for this).
        CTv = CT[:, 0:HB * RW].rearrange("p (h s) -> p h s", s=RW)
        nc.scalar.copy(CTv[:, :, 2:2 + Wo], psv)

        # ---- W^T: 12 tiny transpose-matmuls, q-shift in the lhsT view ----
        # pst_jb[(dh2, w), z] += CT[(q,z), flat (dh2*64 + w - q + 2)] @ Jq
        # where Jq = eye(96)[:, 32q:32q+32] masks the q block. Separate
        # PSUM tiles per jb so each block's WsB cast (scalar) overlaps the
        # next block's matmuls instead of waiting on the whole phase.
        WsB = []
        for jb in range(4):
            pst = psT.tile([128, Z], F32, tag=f"pst{jb}", name=f"pst{jb}")
            for q in range(3):
                base = 128 * jb + 2 - q
                nc.tensor.matmul(pst[:],
                                 CT[0:96, base:base + 128],
                                 Jt[0:96, 32 * q:32 * (q + 1)],
                                 start=(q == 0), stop=(q == 2))
            wsb = big.tile([128, Z], BF16, tag=f"WsB{jb}", name=f"WsB{jb}")
            nc.scalar.copy(wsb[:], pst[:])
            WsB.append(wsb)

        # ---- final: out^T[o, b] += W_chunk.T @ x^T_chunk over 12 chunks --
        pf = psF.tile([O, B_FULL], F32, tag="psF")
        xoff = [0]
        for n in XSPLIT:
            xoff.append(xoff[-1] + n)
        for t12 in range(12):
            c, jb = divmod(t12, 4)
            j = next(i for i in range(len(XSPLIT)) if xoff[i + 1] > t12)
            nc.tensor.matmul(pf[:],
                             WsB[jb][:, O * c:O * (c + 1)],
                             XT[j][:, t12 - xoff[j], :],
                             start=(t12 == 0), stop=(t12 == 11))
        # bf16 output: halves copy+DMA cost; host upcasts (error budget ok)
        obuf = const.tile([O, B_FULL], BF16)
        nc.scalar.copy(obuf[:], pf[:])
        nc.sync.dma_start(o_d, obuf[:])
    nc.compile()
    return nc


def _get_nc():
    global _NC
    if _NC is None:
        _NC = _build()
    return _NC


def _permute_vw(a):
    """(M, 27, 10) fp32 -> [128, 4, 288] bf16 in (p, q, z=c*10+o) layout."""
    ar = a.reshape(M, 3, 3, 3, O)            # (m, c, p, q, o)
    at = ar.transpose(0, 2, 3, 1, 4).reshape(M, 3, 3, 3 * O)
    ap = np.zeros((M, 3, 3, Z), np.float32)
    ap[..., :3 * O] = at
    return ap.reshape(4, 128, KO2).transpose(1, 0, 2).astype(BF16NP)


def _shard_inputs(inputs):
    x = np.ascontiguousarray(inputs["x"], dtype=np.float32)   # (512,3,64,64)
    G = np.ascontiguousarray(inputs["G"], dtype=np.float32)   # (512,3844)
    pd = (np.asarray(inputs["v"], dtype=np.float32)
          - np.asarray(inputs["w"], dtype=np.float32))
    vw = _permute_vw(pd).reshape(128, VW_LEN)                 # [128, 1152]
    Jp = np.zeros((128, 3 * Z), np.float32)
    Jp[:96] = np.eye(96, dtype=np.float32)
    vwj = np.ascontiguousarray(
        np.concatenate([vw, Jp.astype(BF16NP)], axis=1))      # [128, 1248]
    Gim = G.reshape(M, Ho, Wo)
    in_maps = []
    for i in range(N_CORES):
        h0 = HB * i
        xb = x[:, :, h0:h0 + HB, :].reshape(B_FULL, BAND)
        xt = np.ascontiguousarray(xb.T).reshape(12, 128, B_FULL)
        xt = np.ascontiguousarray(
            xt.transpose(1, 0, 2)).reshape(128, 12 * B_FULL).astype(BF16NP)
        gsh = np.zeros((M, IW, Wo), np.float32)
        lo, hi = h0 - 2, h0 + HB          # patch-grid rows needed
        clo, chi = max(lo, 0), min(hi, Ho)
        gsh[:, clo - lo:chi - lo, :] = Gim[:, clo:chi, :]
        gb = gsh.reshape(4, 128, NL).transpose(1, 0, 2).reshape(
            128, 4 * NL).astype(BF16NP)
        gv = np.ascontiguousarray(
            np.concatenate([vwj, gb[:, :2 * NL]], axis=1))    # [128, 2488]
        g2 = np.ascontiguousarray(gb[:, 2 * NL:])             # [128, 1240]
        in_maps.append({"xt": xt, "gv": gv, "g2": g2})
    return in_maps


def _run(inputs, trace=False, **kw):
    nc = _get_nc()
    in_maps = _shard_inputs(inputs)
    res = run_bass_kernel_spmd(nc, in_maps, list(range(N_CORES)),
                               trace=trace, **kw)
    acc = np.zeros((O, B_FULL), np.float64)
    for i in range(N_CORES):
        acc += np.asarray(res.results[i]["out"]).astype(np.float64)
    return np.ascontiguousarray(acc.T).astype(np.float32), res


def kernel(**inputs) -> np.ndarray:
    return _run(inputs)[0]


# revision 34
# speedup vs baseline: 1.0255x; 1.0255x over previous
"""Trainium2 Bass kernel for nn_ConvexReLUCNN.

Math (identical multilinear form as the reference, reordered):
    reference:  U = unfold(x,3); A = U.G^T (54 GFLOP); out = A.(v-w)
    here:       CS[(q,z),(dh,s)] = sum_{m,p} pd[m,(p,q,z)] * G[m,(dh+2-p,s)]
                     (one 12-matmul PSUM accumulation; i-shift absorbed in
                      shifted G windows, p-sum absorbed in the accumulation)
                W^T[(dh,w), z]   = sum_q CS[(q,z), (dh, w-q)]
                     (12 tiny transpose-matmuls vs block identity J;
                      q-shift absorbed in zero-padded CT column views)
                out^T[o, b]      = sum_chunks W_chunk^T @ x^T_chunk

Distribution: sharded by image row band. Core i owns output-image rows
h in [8i, 8i+8) (all channels, widths, batches); host sums the 8 partial
outputs (tiny) - no device collectives.

Schedule: DMA FIFO = (pd|J|g_t01) | g_t23 | x in 4 chunks [5,4,2,1], so
CS starts as the first G half lands and the final matmuls chase the x
chunks, with a tiny last chunk to shorten the x-gated tail. PE junk
warmup (12 wide + 3 taper) bridges the G wait - sized for slow-DMA runs
(cross-core HBM contention varies run to run) - so the HAM clock gate is
at full rate when CS starts - any PE gap >0.5us mid-kernel downclocks
the PE to 1.2 GHz for several microseconds. ONE 96-partition ACT cast
moves CS psum->SBUF (DVE psum-fp32->bf16 casts corrupt on HW). The
transpose phase uses 4 separate PSUM tiles so each jb block's WsB cast
(ACT) overlaps the next block's matmuls. All matmul operands sit at
SBUF base partition 0 (non-zero base partitions hang real HW), with
eye(96) column blocks masking the q selection. bf16 output, host
upcasts and sums partials in float64.

All wire data is bf16, prepared host-side:
  - xt:  x band pre-transposed to [chw=1536, b=512] -> [128, 12*512]
  - gv:  pd=(v-w) pre-permuted to the padded (p,q,z=c*10+o) layout
         [128, 1152], block identity J = eye(96), G band rows t=0,1
  - g2:  G band rows t=2,3 (zero-clipped halo)
"""

import numpy as np
from contextlib import ExitStack

import ml_dtypes

import concourse.bass as bass
import concourse.mybir as mybir
import concourse.tile as tile
from concourse import bacc
from concourse.bass_utils import run_bass_kernel_spmd

N_CORES = 8
B_FULL = 512
C_CH, H, W = 3, 64, 64
HB = H // N_CORES           # 8 image rows per core
BAND = C_CH * HB * W        # 1536 chw positions per core
M = 512                     # num_neurons
O = 10
Ho = Wo = 62
IW = HB + 2                 # 10 patch-grid rows feeding one band
NL = IW * Wo                # 620 local G columns
Z = 32                      # padded (c,o) block: 3*10 -> 32
KO2 = 9 * Z                 # 288 = 3p x 3q x 32z
NW = HB * Wo                # 496 = shifted-G window (8 rows x 62)
RW = 64                     # CT row width: payload s=0..62 at cols 2..64,
                            # so the per-q shifted [96,128] lhsT window is a
                            # flat 1D slice (row wrap hits border zeros)
CTS_LEN = HB * RW + 2       # 514 (+2 tail zeros for the q=0 wrap past row 7)
VW_LEN = 4 * KO2            # 1152 (host-subtracted pd = v - w)
VWJ_LEN = VW_LEN + 3 * Z    # 1248 (+ block identity J = eye(96))
GV_LEN = VWJ_LEN + 2 * NL   # 2488 (pd | J | g t=0,1)
XSPLIT = [5, 4, 2, 1]       # t12 tiles per x chunk: small late chunks so
                            # the x-gated tail of the final matmuls is short
NWARM_WIDE = 12             # wide (512-col) PE warmup matmuls
NWARM_TAPER = 3             # narrow (128-col) taper warmups

F32 = mybir.dt.float32
BF16 = mybir.dt.bfloat16
BF16NP = ml_dtypes.bfloat16

_NC = None


def _build():
    nc = bacc.Bacc("TRN2", target_bir_lowering=False, debug=False,
                   num_devices=N_CORES)
    x_d = nc.dram_tensor("xt", [128, 12 * B_FULL], BF16,
                         kind="ExternalInput").ap()
    gv_d = nc.dram_tensor("gv", [128, GV_LEN], BF16,
                          kind="ExternalInput").ap()
    g2_d = nc.dram_tensor("g2", [128, 2 * NL], BF16,
                          kind="ExternalInput").ap()
    o_d = nc.dram_tensor("out", [O, B_FULL], BF16, kind="ExternalOutput").ap()

    with tile.TileContext(nc) as tc, ExitStack() as ctx:
        const = ctx.enter_context(tc.tile_pool(name="const", bufs=1))
        big = ctx.enter_context(tc.tile_pool(name="big", bufs=1))
        psW = ctx.enter_context(tc.tile_pool(name="psW", bufs=1, space="PSUM"))
        psC = ctx.enter_context(tc.tile_pool(name="psC", bufs=1, space="PSUM"))
        psT = ctx.enter_context(tc.tile_pool(name="psT", bufs=1, space="PSUM"))
        psF = ctx.enter_context(tc.tile_pool(name="psF", bufs=1, space="PSUM"))

        # ---- DMA triggers first: all on the sync HWDGE ring, FIFO --------
        # FIFO order == consumption order: (pd|J|g_t0), g_t1, g_t23, x.
        gvt = big.tile([128, GV_LEN], BF16, tag="gvt")
        nc.sync.dma_start(gvt[:], gv_d)
        g2t = big.tile([128, 2, NL], BF16, tag="g2t")
        nc.sync.dma_start(g2t[:], g2_d)
        XT = [big.tile([128, n, B_FULL], BF16, tag=f"X{j}", name=f"X{j}")
              for j, n in enumerate(XSPLIT)]
        w0 = 0
        for j, n in enumerate(XSPLIT):
            nc.sync.dma_start(XT[j][:], x_d[:, w0:w0 + n * B_FULL])
            w0 += n * B_FULL

        # ---- memsets: junk on GpSimd (earliest engine up -> PE warmup
        # starts ~1.5us sooner, so the HAM clock gate hits full rate by CS)
        junk = const.tile([128, 512], BF16)
        nc.gpsimd.memset(junk[:], 0.25)
        # CT: rows (q,z), cols (dh, 2+s) with 64-wide rows, zero borders
        CT = const.tile([96, CTS_LEN], BF16, name="CT")
        nc.vector.memset(CT[:], 0.0)

        # ---- PE p-state warmup on junk (bridges the vwj/g1 DMA wait) -----
        pj = psW.tile([128, 512], F32, tag="pj")
        for _ in range(NWARM_WIDE):
            nc.tensor.matmul(pj[:], junk[:, 0:128], junk[:, 0:512],
                             start=True, stop=True)
        for _ in range(NWARM_TAPER):
            nc.tensor.matmul(pj[:, 0:128], junk[:, 0:128], junk[:, 0:128],
                             start=True, stop=True)

        # pd = v - w arrives host-subtracted in the padded (p, q, z) layout
        pd2 = gvt[:, 0:VW_LEN].rearrange("p (t k) -> p t k", t=4)
        Jt = gvt[:, VW_LEN:VWJ_LEN]             # [96 used, 96] = eye(96)
        gs1 = gvt[:, VWJ_LEN:GV_LEN].rearrange("p (t k) -> p t k", t=2)

        # ---- CS = sum_{t,p} pd2_tp.T @ G_t[rows 2-p .. 10-p] -------------
        ps = psC.tile([96, NW], F32, tag="psC")
        gblks = [gs1[:, 0, :], gs1[:, 1, :], g2t[:, 0, :], g2t[:, 1, :]]
        for t in range(4):
            gblk = gblks[t]
            for p in range(3):
                nc.tensor.matmul(
                    ps[:],
                    pd2[:, t, 96 * p:96 * (p + 1)],
                    gblk[:, Wo * (2 - p):Wo * (2 - p) + NW],
                    start=(t == 0 and p == 0), stop=(t == 3 and p == 2))
        psv = ps[:].rearrange("p (h s) -> p h s", s=Wo)
        # ONE 96-partition psum->SBUF cast (DVE psum-fp32->bf16 corrupts on
        # HW; ACT is the safe engine for this).
        CTv = CT[:, 0:HB * RW].rearrange("p (h s) -> p h s", s=RW)
        nc.scalar.copy(CTv[:, :, 2:2 + Wo], psv)

        # ---- W^T: 12 tiny transpose-matmuls, q-shift in the lhsT view ----
        # pst_jb[(dh2, w), z] += CT[(q,z), flat (dh2*64 + w - q + 2)] @ Jq
        # where Jq = eye(96)[:, 32q:32q+32] masks the q block. Separate
        # PSUM tiles per jb so each block's WsB cast (scalar) overlaps the
        # next block's matmuls instead of waiting on the whole phase.
        WsB = []
        for jb in range(4):
            pst = psT.tile([128, Z], F32, tag=f"pst{jb}", name=f"pst{jb}")
            for q in range(3):
                base = 128 * jb + 2 - q
                nc.tensor.matmul(pst[:],
                                 CT[0:96, base:base + 128],
                                 Jt[0:96, 32 * q:32 * (q + 1)],
                                 start=(q == 0), stop=(q == 2))
            wsb = big.tile([128, Z], BF16, tag=f"WsB{jb}", name=f"WsB{jb}")
            nc.scalar.copy(wsb[:], pst[:])
            WsB.append(wsb)

        # ---- final: out^T[o, b] += W_chunk.T @ x^T_chunk over 12 chunks --
        pf = psF.tile([O, B_FULL], F32, tag="psF")
        xoff = [0]
        for n in XSPLIT:
            xoff.append(xoff[-1] + n)
        for t12 in range(12):
            c, jb = divmod(t12, 4)
            j = next(i for i in range(len(XSPLIT)) if xoff[i + 1] > t12)
            nc.tensor.matmul(pf[:],
                             WsB[jb][:, O * c:O * (c + 1)],
                             XT[j][:, t12 - xoff[j], :],
                             start=(t12 == 0), stop=(t12 == 11))
        # bf16 output: halves copy+DMA cost; host upcasts (error budget ok)
        obuf = const.tile([O, B_FULL], BF16)
        nc.scalar.copy(obuf[:], pf[:])
        nc.sync.dma_start(o_d, obuf[:])
    nc.compile()
    return nc


def _get_nc():
    global _NC
    if _NC is None:
        _NC = _build()
    return _NC


def _permute_vw(a):
    """(M, 27, 10) fp32 -> [128, 4, 288] bf16 in (p, q, z=c*10+o) layout."""
    ar = a.reshape(M, 3, 3, 3, O)            # (m, c, p, q, o)
    at = ar.transpose(0, 2, 3, 1, 4).reshape(M, 3, 3, 3 * O)
    ap = np.zeros((M, 3, 3, Z), np.float32)
    ap[..., :3 * O] = at
    return ap.reshape(4, 128, KO2).transpose(1, 0, 2).astype(BF16NP)


def _shard_inputs(inputs):
    x = np.ascontiguousarray(inputs["x"], dtype=np.float32)   # (512,3,64,64)
    G = np.ascontiguousarray(inputs["G"], dtype=np.float32)   # (512,3844)
    pd = (np.asarray(inputs["v"], dtype=np.float32)
          - np.asarray(inputs["w"], dtype=np.float32))
    vw = _permute_vw(pd).reshape(128, VW_LEN)                 # [128, 1152]
    Jp = np.zeros((128, 3 * Z), np.float32)
    Jp[:96] = np.eye(96, dtype=np.float32)
    vwj = np.ascontiguousarray(
        np.concatenate([vw, Jp.astype(BF16NP)], axis=1))      # [128, 1248]
    Gim = G.reshape(M, Ho, Wo)
    in_maps = []
    for i in range(N_CORES):
        h0 = HB * i
        xb = x[:, :, h0:h0 + HB, :].reshape(B_FULL, BAND)
        xt = np.ascontiguousarray(xb.T).reshape(12, 128, B_FULL)
        xt = np.ascontiguousarray(
            xt.transpose(1, 0, 2)).reshape(128, 12 * B_FULL).astype(BF16NP)
        gsh = np.zeros((M, IW, Wo), np.float32)
        lo, hi = h0 - 2, h0 + HB          # patch-grid rows needed
        clo, chi = max(lo, 0), min(hi, Ho)
        gsh[:, clo - lo:chi - lo, :] = Gim[:, clo:chi, :]
        gb = gsh.reshape(4, 128, NL).transpose(1, 0, 2).reshape(
            128, 4 * NL).astype(BF16NP)
        gv = np.ascontiguousarray(
            np.concatenate([vwj, gb[:, :2 * NL]], axis=1))    # [128, 2488]
        g2 = np.ascontiguousarray(gb[:, 2 * NL:])             # [128, 1240]
        in_maps.append({"xt": xt, "gv": gv, "g2": g2})
    return in_maps


def _run(inputs, trace=False, **kw):
    nc = _get_nc()
    in_maps = _shard_inputs(inputs)
    res = run_bass_kernel_spmd(nc, in_maps, list(range(N_CORES)),
                               trace=trace, **kw)
    acc = np.zeros((O, B_FULL), np.float64)
    for i in range(N_CORES):
        acc += np.asarray(res.results[i]["out"]).astype(np.float64)
    return np.ascontiguousarray(acc.T).astype(np.float32), res


def kernel(**inputs) -> np.ndarray:
    return _run(inputs)[0]
